# revision 31
# baseline (speedup 1.0000x reference)
"""Trainium2 Bass kernel for a BiQRNN3D layer.

reference math:
  gates = conv3d(x, W, SAME, 3x3x3) + b          x: [2,16,31,256,256] f32
  Z, F1, F2 = split(gates, 3, channel)           W: [48,16,3,3,3], b: [48]
  Z = tanh(Z); F1 = sigmoid(F1); F2 = sigmoid(F2)
  h_fwd: depth-forward  recurrence h = F1*h + (1-F1)*Z
  h_bwd: depth-backward recurrence h = F2*h + (1-F2)*Z
  out = h_fwd + h_bwd                            [2,16,31,256,256] f32

Distribution: H (=256) is sharded 32 rows per core across 8 NeuronCores
(SPMD, identical program; each core's x shard carries its 1-row conv halo
with global-edge zeros baked in by the host).

Per-core pipeline (gates never leave the chip; the v1 baseline spilled
them to DRAM and re-read via XBAR DMA-transpose at ~37 GB/s, which
saturated all 16 DMA queues at a ~41us h-block period / 1.32 ms total):
  * conv as matmul, K = (kd,ci) = 48 contraction rows. The moving x
    half-buffers each hold ONE h-plane pair (A rows: plane 2q-1, B rows:
    plane 2q, 3 kd-shifted copies per plane); consecutive h-blocks share
    a pair, so each block DMAs only one new half (halves x traffic to
    ~49 MB/core). Partition 48 is a ones-row (bias rides as a stationary
    row); 49-63 are zeros.
  * M = 96: stationary columns (j, co) produce BOTH output h rows of an
    h-block at once. Per psum tile [96, 2*256] six K=112 matmuls
    accumulate: passes (p in {0,1}) x (kw in {0,1,2}). This is the
    roofline: 6 passes x 512 cols x 16 d-tiles x 32 blocks ~= 650us of
    PE stream at ~218 ns / 512-col matmul (warm p-state).
  * F1/F2 stationary columns (weights AND bias) pre-scaled by 0.5 on the
    host, so ONE Tanh activation per psum tile evacuates PSUM and applies
    all three nonlinearities into ev [128, D, W] fp16 (rows 96:128 are
    zeroed once so the K=128 transpose loads below see no garbage).
  * Transpose on the PE as a REGULAR matmul against a [128, 96] identity
    (data as stationary): per (d, w-chunk), ev[:, d, wc*128:+128] becomes
    PSUM [128 px, 96 (j,co)] f32.  The psum dst AP is d-strided so a
    group of 5 d-slices lands as (j, co, d) column order in one bank;
    the group evac (DVE tensor_copy for wc0 / ACT Copy for wc1, split to
    balance engines) and the scans are then contiguous.  is_transpose
    mode is NOT used: it skips the warm p-state and costs ~275 ns vs
    ~84 ns here.  Each transpose lags its conv tile by 2 so its
    LDWEIGHTS (~107 ns) hides under a 218 ns conv stream; the lag queue
    persists across h-blocks (block tails interleave into the next
    block's conv stream) and scans/stores are emitted when a block's
    last evac retires.  PSUM budget: 3 conv banks + 5 transpose banks.
  * DVE per w-chunk on T [128 px, (j, co, d)]: f = (t+1)/2 and
    g' = (t-1)*z (2x the true g; the recurrence is linear in g so
    h' = 2h and the host halves it), tensor_tensor_scan (h = f*h - g)
    forward over the flattened (j, hid, d) runs with f zeroed at each
    run's first d, and backward via fully-reversed APs; o = h'_fwd +
    h'_bwd into ob; ONE DMA per h-block stores to out [S, HID, D].

Measured (8 cores, SPMD): 878 us vs 1324 us baseline (1.51x), PE busy
~93% with no >50ns gaps; DVE ~74%, ACT ~55%, DMA queues ~45%.  gpsimd
compute offload was tried and reverted (its ADD is 4x slower than DVE
and sat on the ob critical path).
"""

from contextlib import ExitStack

import numpy as np

import concourse.bass as bass
import concourse.tile as tile
from concourse import bacc, mybir

F32 = mybir.dt.float32
F16 = mybir.dt.float16
AF = mybir.ActivationFunctionType
ALU = mybir.AluOpType

N_CORES = 8
B = 2
CIN = 16
HID = 16
CO = 3 * HID            # 48
D = 31
H = 256
W = 256
HSH = H // N_CORES      # 32
HB = 2                  # output h rows per conv tile (= M/CO)
DC = 2                  # d slices per psum tile
WP = W + 2
S = B * HSH * W         # 16384
FX = D * WP             # x half-tile free extent per partition
NST = 6                 # stationary matrices
NBLK = B * (HSH // HB)  # 32 h-blocks per core
NHALF = HSH // HB + 1   # 17 x half-tiles per batch
CD = CO * D             # 1488
DG = 5                  # d-slices per transpose psum group (f32 bank limit)
NXS = 4                 # rotating x half-buffer slots


def _build_program(reps=1):
    nc = bacc.Bacc("TRN2", target_bir_lowering=False, debug=False)

    xhalf = nc.dram_tensor("x", [B * NHALF, 96, D, WP], F16,
                           kind="ExternalInput").ap()
    wts = nc.dram_tensor("wts", [128, NST * 2 * CO], F16,
                         kind="ExternalInput").ap()
    aux = nc.dram_tensor("aux", [16, FX], F16, kind="ExternalInput").ap()
    ident = nc.dram_tensor("ident", [128, 96], F16,
                           kind="ExternalInput").ap()
    out = nc.dram_tensor("out", [S, HID, D], F16, kind="ExternalOutput").ap()

    with tile.TileContext(nc) as tc, ExitStack() as ctx:
        wsb = nc.alloc_sbuf_tensor("wsb", [128, NST * 2 * CO], F16).ap()
        isb = nc.alloc_sbuf_tensor("isb", [128, 96], F16).ap()
        # x half-buffers: slot s holds ONE plane-group; a block's pass p
        # streams slot (hid+p) % NXS.  A rows: plane 2q-1, B rows: plane 2q.
        xbufs = [nc.alloc_sbuf_tensor(f"xb{i}", [112, D, WP], F16).ap()
                 for i in range(NXS)]
        # ev is static (not pooled) so rows 96:128 can be zeroed ONCE and
        # serve as FWL padding for the K=128 transpose stationary loads
        evbufs = [nc.alloc_sbuf_tensor(f"ev{i}", [128, D, W], F16).ap()
                  for i in range(2)]

        nc.sync.dma_start(wsb, wts)
        nc.sync.dma_start(isb, ident)
        for xb in xbufs:
            nc.sync.dma_start(
                xb[48:64].rearrange("p a b -> p (a b)"), aux)
        for evb in evbufs:
            nc.vector.memset(
                evb[96:128].rearrange("p a b -> p (a b)"), 0.0)

        ps_pool = ctx.enter_context(tc.tile_pool(name="ps", bufs=3,
                                                 space="PSUM"))
        tp_pool = ctx.enter_context(tc.tile_pool(name="tp", bufs=5,
                                                 space="PSUM"))
        t_pool = ctx.enter_context(tc.tile_pool(name="tt", bufs=4))
        sc_pool = ctx.enter_context(tc.tile_pool(name="sc", bufs=2))
        ob_pool = ctx.enter_context(tc.tile_pool(name="ob", bufs=2))

        n_dc = (D + DC - 1) // DC       # 16
        n_grp = (D + DG - 1) // DG      # 4

        def scan_chunk(T, ob, wc):
            # T: [128 px, (j, co, d)] fp16, co = gate*16+hid
            Tv = T[:].rearrange("p (j c d) -> p j c d", j=HB, c=CO)
            Tz = Tv[:, :, 0:HID]
            T1 = Tv[:, :, HID:2 * HID]
            T2 = Tv[:, :, 2 * HID:3 * HID]
            f1 = sc_pool.tile([128, HB * HID * D], F16, tag="f1")
            f2 = sc_pool.tile([128, HB * HID * D], F16, tag="f2")
            g1 = sc_pool.tile([128, HB * HID * D], F16, tag="g1")
            g2 = sc_pool.tile([128, HB * HID * D], F16, tag="g2")
            f1v = f1[:].rearrange("p (j h d) -> p j h d", j=HB, h=HID)
            f2v = f2[:].rearrange("p (j h d) -> p j h d", j=HB, h=HID)
            g1v = g1[:].rearrange("p (j h d) -> p j h d", j=HB, h=HID)
            g2v = g2[:].rearrange("p (j h d) -> p j h d", j=HB, h=HID)
            nc.vector.tensor_scalar(f1v, T1, 0.5, 0.5, ALU.mult, ALU.add)
            nc.vector.tensor_scalar(f2v, T2, 0.5, 0.5, ALU.mult, ALU.add)
            nc.vector.scalar_tensor_tensor(
                g1v, T1, 1.0, Tz, ALU.subtract, ALU.mult)
            nc.vector.scalar_tensor_tensor(
                g2v, T2, 1.0, Tz, ALU.subtract, ALU.mult)
            nc.vector.memset(f1v[:, :, :, 0:1], 0.0)
            nc.vector.memset(f2v[:, :, :, D - 1:D], 0.0)
            h1 = sc_pool.tile([128, HB * HID * D], F16, tag="h1")
            h2 = sc_pool.tile([128, HB * HID * D], F16, tag="h2")
            nc.vector.tensor_tensor_scan(
                h1[:], f1[:], g1[:], 0.0, ALU.mult, ALU.subtract)
            nc.vector.tensor_tensor_scan(
                h2[:][:, ::-1], f2[:][:, ::-1], g2[:][:, ::-1],
                0.0, ALU.mult, ALU.subtract)
            nc.vector.tensor_add(
                ob[:, :, wc],
                h1[:].rearrange("p (j cd) -> p j cd", j=HB),
                h2[:].rearrange("p (j cd) -> p j cd", j=HB))

        def load_half(h):
            xb = xbufs[h % NXS]
            nc.sync.dma_start(xb[0:48], xhalf[h % (B * NHALF), 0:48])
            nc.sync.dma_start(xb[64:112], xhalf[h % (B * NHALF), 48:96])

        def hid_of(blk):
            return (blk // (NBLK // B)) * NHALF + (blk % (NBLK // B))

        nblk_tot = reps * NBLK
        nhid_tot = reps * B * NHALF
        next_hid = 0
        while next_hid <= min(3, nhid_tot - 1):
            load_half(next_hid)
            next_hid += 1
        n_grp_tot = 2 * ((D + DG - 1) // DG)   # evacs per block

        def emit_scans(bc):
            ob = ob_pool.tile([128, HB, 2, HID * D], F16, tag="ob",
                              name="ob")
            for wc in range(2):
                scan_chunk(bc["Ts"][wc], ob, wc)
            dst = out[bc["s0"]:bc["s0"] + HB * W].rearrange(
                "(jq p) c d -> p jq (c d)", p=128)
            nc.gpsimd.dma_start(dst, ob[:].rearrange(
                "p j q cd -> p (j q) cd"))

        def emit_tp(bc, dd, wc):
            # regular matmul against identity: out[w, (j,c)] =
            # ev[(j,c), w]; strided psum dst makes the group's column
            # order (j, c, d) so evac+scan are contiguous
            g = dd // DG
            kk = dd % DG
            tps = bc["tps"]
            if (wc, g) not in tps:
                tps[(wc, g)] = tp_pool.tile(
                    [128, DG * 96], F32, tag="tp", name="tp")
            tpv = tps[(wc, g)][:].rearrange(
                "p (j c d) -> p j c d", j=HB, c=CO)
            nc.tensor.matmul(
                tpv[:, :, :, kk],
                bc["ev"][:, dd, wc * 128:(wc + 1) * 128],
                isb[0:128, 0:96],
                start=True, stop=True)
            if kk == DG - 1 or dd == D - 1:
                dk = kk + 1
                dst = bc["Tvs"][wc][:, :, :, g * DG:g * DG + dk]
                src = tps[(wc, g)][:].rearrange(
                    "p (j c d) -> p j c d", j=HB, c=CO)[:, :, :, 0:dk]
                if wc == 0:
                    nc.vector.tensor_copy(dst, src)
                else:
                    nc.scalar.activation(dst, src, AF.Copy)
                bc["nevac"] += 1
                if bc["nevac"] == n_grp_tot:
                    emit_scans(bc)

        # transposes lag their d-slice by 2 conv tiles (so their
        # LDWEIGHTS hide under conv streams and the tanh is done); the
        # queue persists across blocks so block tails interleave into the
        # next block's conv stream
        pending = []
        for blk in range(nblk_tot):
            hid = hid_of(blk % NBLK) + (blk // NBLK) * B * NHALF
            s0 = (blk % NBLK) * HB * W
            ev = evbufs[blk % 2]
            Ts = [t_pool.tile([128, HB * CD], F16, tag=f"T{wc}",
                              name=f"T{wc}")
                  for wc in range(2)]
            bc = {"ev": ev, "Ts": Ts,
                  "Tvs": [T[:].rearrange("p (j c d) -> p j c d",
                                         j=HB, c=CO) for T in Ts],
                  "tps": {}, "nevac": 0, "s0": s0}
            for dc in range(n_dc):
                gdc = blk * n_dc + dc
                d0 = dc * DC
                dn = min(DC, D - d0)
                ps = ps_pool.tile([2 * CO, DC * W], F32, tag="ps")
                psv = ps[:, 0:dn * W].rearrange("p (d w) -> p d w", w=W)
                k = 0
                for p in range(2):
                    xb = xbufs[(hid + p) % NXS]
                    for kw in range(3):
                        nc.tensor.matmul(
                            psv,
                            wsb[0:112, k * 96:(k + 1) * 96],
                            xb[0:112, d0:d0 + dn, kw:kw + W],
                            start=(k == 0), stop=(k == NST - 1))
                        if k >= 2 and pending and pending[0][0] <= gdc - 2:
                            _, pbc, dd, wc = pending.pop(0)
                            emit_tp(pbc, dd, wc)
                        k += 1
                evv = ev[0:96, d0:d0 + dn, :].rearrange("p d w -> p (d w)")
                nc.scalar.activation(evv, ps[:, 0:dn * W], AF.Tanh)
                for dd in range(d0, d0 + dn):
                    for wc in range(2):
                        pending.append((gdc, bc, dd, wc))
            while next_hid < nhid_tot and next_hid <= hid + 3:
                load_half(next_hid)
                next_hid += 1
        for _, pbc, dd, wc in pending:
            emit_tp(pbc, dd, wc)

    nc.finalize()
    return nc


def _host_inputs(x, Wc, b):
    """x: [B, CIN, D, H, W] f32 full input. Returns list of 8 in_maps."""
    bf = np.float16
    # 6 stationaries: idx = p*3+kw, each [128, 96] with cols (j*48+co).
    # rows 0-47 (block A, x at tile-h 2p):   tap kh = 2p - j
    # rows 64-111 (block B, x at h+1):       tap kh = 2p + 1 - j
    wt = np.zeros((NST, 128, 2 * CO), np.float32)
    for p in range(2):
        for kw in range(3):
            idx = p * 3 + kw
            for j in range(2):
                c0 = j * CO
                for blk, khv in ((0, 2 * p - j), (64, 2 * p + 1 - j)):
                    if khv < 0 or khv > 2:
                        continue
                    for kd in range(3):
                        p0 = blk + kd * 16
                        wt[idx, p0:p0 + 16, c0:c0 + CO] = \
                            Wc[:, :, kd, khv, kw].T
    wt[0, 48, 0:CO] = b
    wt[0, 48, CO:2 * CO] = b
    # pre-scale F1/F2 columns (weights and bias) by 0.5 so the single
    # Tanh evac yields t with sigmoid(a) = (t+1)/2
    for j in range(2):
        wt[:, :, j * CO + HID:j * CO + 3 * HID] *= 0.5
    wts = wt.transpose(1, 0, 2).reshape(128, NST * 2 * CO).astype(bf)
    auxa = np.zeros((16, FX), np.float32)
    auxa[0, :] = 1.0
    auxa = auxa.astype(bf)
    identa = np.zeros((128, 96), bf)
    identa[0:96, 0:96] = np.eye(96, dtype=bf)

    xt = np.ascontiguousarray(x.transpose(1, 2, 0, 3, 4)).astype(bf)
    in_maps = []
    for c in range(N_CORES):
        hs, he = c * HSH, (c + 1) * HSH
        xp = np.zeros((CIN, D + 2, B, HSH + 2, WP), bf)
        lo = max(hs - 1, 0)
        hi = min(he + 1, H)
        xp[:, 1:D + 1, :, (lo - (hs - 1)):(hi - (hs - 1)), 1:W + 1] = \
            xt[:, :, :, lo:hi, :]
        # pack x half-tiles: [B*NHALF, 96, D, WP], half q of batch b:
        # rows kd*16+ci    = xp[ci, kd+d, b, 2q, w]     (A: plane 2q-1)
        # rows 48+kd*16+ci = xp[ci, kd+d, b, 2q+1, w]   (B: plane 2q)
        xbk = np.empty((B, NHALF, 96, D, WP), bf)
        for kd in range(3):
            sl = xp[:, kd:kd + D]            # [CIN, D, B, HSH+2, WP]
            qa = np.arange(NHALF) * 2
            arr = sl[:, :, :, qa, :].transpose(2, 3, 0, 1, 4)
            xbk[:, :, kd * 16:kd * 16 + 16] = arr
            arr = sl[:, :, :, qa + 1, :].transpose(2, 3, 0, 1, 4)
            xbk[:, :, 48 + kd * 16:48 + kd * 16 + 16] = arr
        xbk = xbk.reshape(B * NHALF, 96, D, WP)
        in_maps.append({"x": xbk, "wts": wts, "aux": auxa, "ident": identa})
    return in_maps


_PROGRAM = None


def _get_program():
    global _PROGRAM
    if _PROGRAM is None:
        _PROGRAM = _build_program()
    return _PROGRAM


def run_sharded(in_maps, trace=False, **kw):
    from concourse import bass_utils
    nc = _get_program()
    return bass_utils.run_bass_kernel_spmd(
        nc, in_maps, core_ids=list(range(N_CORES)), trace=trace, **kw)


def _assemble(results):
    outf = np.empty((B, HID, D, H, W), np.float32)
    for c in range(N_CORES):
        raw = np.asarray(results[c]["out"]).astype(np.float32) * 0.5
        o = raw.reshape(B, HSH, W, HID, D).transpose(0, 3, 4, 1, 2)
        outf[:, :, :, c * HSH:(c + 1) * HSH, :] = o
    return outf


def kernel(x, W, b):
    x = np.asarray(x, np.float32)
    W = np.asarray(W, np.float32)
    b = np.asarray(b, np.float32)
    in_maps = _host_inputs(x, W, b)
    res = run_sharded(in_maps)
    return _assemble(res.results)


# revision 34
# speedup vs baseline: 1.0001x; 1.0001x over previous
"""Trainium2 Bass kernel for a BiQRNN3D layer.

reference math:
  gates = conv3d(x, W, SAME, 3x3x3) + b          x: [2,16,31,256,256] f32
  Z, F1, F2 = split(gates, 3, channel)           W: [48,16,3,3,3], b: [48]
  Z = tanh(Z); F1 = sigmoid(F1); F2 = sigmoid(F2)
  h_fwd: depth-forward  recurrence h = F1*h + (1-F1)*Z
  h_bwd: depth-backward recurrence h = F2*h + (1-F2)*Z
  out = h_fwd + h_bwd                            [2,16,31,256,256] f32

Distribution: H (=256) is sharded 32 rows per core across 8 NeuronCores
(SPMD, identical program; each core's x shard carries its 1-row conv halo
with global-edge zeros baked in by the host).

Per-core pipeline (gates never leave the chip; the v1 baseline spilled
them to DRAM and re-read via XBAR DMA-transpose at ~37 GB/s, which
saturated all 16 DMA queues at a ~41us h-block period / 1.32 ms total):
  * conv as matmul, K = (kd,ci) = 48 contraction rows. The moving x
    half-buffers each hold ONE h-plane pair (A rows: plane 2q-1, B rows:
    plane 2q, 3 kd-shifted copies per plane); consecutive h-blocks share
    a pair, so each block DMAs only one new half (halves x traffic to
    ~49 MB/core). Partition 48 is a ones-row (bias rides as a stationary
    row); 49-63 are zeros.
  * M = 96: stationary columns (j, co) produce BOTH output h rows of an
    h-block at once. Per psum tile [96, 2*256] six K=112 matmuls
    accumulate: passes (p in {0,1}) x (kw in {0,1,2}). This is the
    roofline: 6 passes x 512 cols x 16 d-tiles x 32 blocks ~= 650us of
    PE stream at ~218 ns / 512-col matmul (warm p-state).
  * F1/F2 stationary columns (weights AND bias) pre-scaled by 0.5 on the
    host, so ONE Tanh activation per psum tile evacuates PSUM and applies
    all three nonlinearities into ev [128, D, W] fp16 (rows 96:128 are
    zeroed once so the K=128 transpose loads below see no garbage).
  * Transpose on the PE as a REGULAR matmul against a [128, 96] identity
    (data as stationary): per (d, w-chunk), ev[:, d, wc*128:+128] becomes
    PSUM [128 px, 96 (j,co)] f32.  The psum dst AP is d-strided so a
    group of 5 d-slices lands as (j, co, d) column order in one bank;
    the group evac (DVE tensor_copy for wc0 / ACT Copy for wc1, split to
    balance engines) and the scans are then contiguous.  is_transpose
    mode is NOT used: it skips the warm p-state and costs ~275 ns vs
    ~84 ns here.  Each transpose lags its conv tile by 2 so its
    LDWEIGHTS (~107 ns) hides under a 218 ns conv stream; the lag queue
    persists across h-blocks (block tails interleave into the next
    block's conv stream) and scans/stores are emitted when a block's
    last evac retires.  PSUM budget: 3 conv banks + 5 transpose banks.
  * DVE per w-chunk on T [128 px, (j, co, d)]: f = (t+1)/2 and
    g' = (t-1)*z (2x the true g; the recurrence is linear in g so
    h' = 2h and the host halves it), tensor_tensor_scan (h = f*h - g)
    forward over the flattened (j, hid, d) runs with f zeroed at each
    run's first d, and backward via fully-reversed APs; o = h'_fwd +
    h'_bwd into ob; ONE DMA per h-block stores to out [S, HID, D].

Measured (8 cores, SPMD): 878 us vs 1324 us baseline (1.51x), PE busy
~93% with no >50ns gaps; DVE ~74%, ACT ~55%, DMA queues ~45%.  gpsimd
compute offload was tried and reverted (its ADD is 4x slower than DVE
and sat on the ob critical path).
"""

from contextlib import ExitStack

import numpy as np

import concourse.bass as bass
import concourse.tile as tile
from concourse import bacc, mybir

F32 = mybir.dt.float32
F16 = mybir.dt.float16
AF = mybir.ActivationFunctionType
ALU = mybir.AluOpType

N_CORES = 8
B = 2
CIN = 16
HID = 16
CO = 3 * HID            # 48
D = 31
H = 256
W = 256
HSH = H // N_CORES      # 32
HB = 2                  # output h rows per conv tile (= M/CO)
DC = 2                  # d slices per psum tile
WP = W + 2
S = B * HSH * W         # 16384
FX = D * WP             # x half-tile free extent per partition
NST = 6                 # stationary matrices
NBLK = B * (HSH // HB)  # 32 h-blocks per core
NHALF = HSH // HB + 1   # 17 x half-tiles per batch
CD = CO * D             # 1488
DG = 5                  # d-slices per transpose psum group (f32 bank limit)
NXS = 4                 # rotating x half-buffer slots


def _build_program(reps=1):
    nc = bacc.Bacc("TRN2", target_bir_lowering=False, debug=False)

    xhalf = nc.dram_tensor("x", [B * NHALF, 96, D, WP], F16,
                           kind="ExternalInput").ap()
    wts = nc.dram_tensor("wts", [128, NST * 2 * CO], F16,
                         kind="ExternalInput").ap()
    aux = nc.dram_tensor("aux", [16, FX], F16, kind="ExternalInput").ap()
    ident = nc.dram_tensor("ident", [128, 96], F16,
                           kind="ExternalInput").ap()
    out = nc.dram_tensor("out", [S, HID, D], F16, kind="ExternalOutput").ap()

    with tile.TileContext(nc) as tc, ExitStack() as ctx:
        wsb = nc.alloc_sbuf_tensor("wsb", [128, NST * 2 * CO], F16).ap()
        isb = nc.alloc_sbuf_tensor("isb", [128, 96], F16).ap()
        # x half-buffers: slot s holds ONE plane-group; a block's pass p
        # streams slot (hid+p) % NXS.  A rows: plane 2q-1, B rows: plane 2q.
        xbufs = [nc.alloc_sbuf_tensor(f"xb{i}", [112, D, WP], F16).ap()
                 for i in range(NXS)]
        # ev is static (not pooled) so rows 96:128 can be zeroed ONCE and
        # serve as FWL padding for the K=128 transpose stationary loads
        evbufs = [nc.alloc_sbuf_tensor(f"ev{i}", [128, D, W], F16).ap()
                  for i in range(2)]

        nc.sync.dma_start(wsb, wts)
        nc.sync.dma_start(isb, ident)
        for xb in xbufs:
            nc.sync.dma_start(
                xb[48:64].rearrange("p a b -> p (a b)"), aux)
        for evb in evbufs:
            nc.vector.memset(
                evb[96:128].rearrange("p a b -> p (a b)"), 0.0)

        ps_pool = ctx.enter_context(tc.tile_pool(name="ps", bufs=3,
                                                 space="PSUM"))
        tp_pool = ctx.enter_context(tc.tile_pool(name="tp", bufs=5,
                                                 space="PSUM"))
        t_pool = ctx.enter_context(tc.tile_pool(name="tt", bufs=4))
        sc_pool = ctx.enter_context(tc.tile_pool(name="sc", bufs=2))
        ob_pool = ctx.enter_context(tc.tile_pool(name="ob", bufs=2))

        n_dc = (D + DC - 1) // DC       # 16
        n_grp = (D + DG - 1) // DG      # 4

        def scan_chunk(T, ob, wc):
            # T: [128 px, (j, co, d)] fp16, co = gate*16+hid
            Tv = T[:].rearrange("p (j c d) -> p j c d", j=HB, c=CO)
            Tz = Tv[:, :, 0:HID]
            T1 = Tv[:, :, HID:2 * HID]
            T2 = Tv[:, :, 2 * HID:3 * HID]
            f1 = sc_pool.tile([128, HB * HID * D], F16, tag="f1")
            f2 = sc_pool.tile([128, HB * HID * D], F16, tag="f2")
            g1 = sc_pool.tile([128, HB * HID * D], F16, tag="g1")
            g2 = sc_pool.tile([128, HB * HID * D], F16, tag="g2")
            f1v = f1[:].rearrange("p (j h d) -> p j h d", j=HB, h=HID)
            f2v = f2[:].rearrange("p (j h d) -> p j h d", j=HB, h=HID)
            g1v = g1[:].rearrange("p (j h d) -> p j h d", j=HB, h=HID)
            g2v = g2[:].rearrange("p (j h d) -> p j h d", j=HB, h=HID)
            nc.vector.tensor_scalar(f1v, T1, 0.5, 0.5, ALU.mult, ALU.add)
            nc.vector.tensor_scalar(f2v, T2, 0.5, 0.5, ALU.mult, ALU.add)
            nc.vector.scalar_tensor_tensor(
                g1v, T1, 1.0, Tz, ALU.subtract, ALU.mult)
            nc.vector.scalar_tensor_tensor(
                g2v, T2, 1.0, Tz, ALU.subtract, ALU.mult)
            nc.vector.memset(f1v[:, :, :, 0:1], 0.0)
            nc.vector.memset(f2v[:, :, :, D - 1:D], 0.0)
            h1 = sc_pool.tile([128, HB * HID * D], F16, tag="h1")
            h2 = sc_pool.tile([128, HB * HID * D], F16, tag="h2")
            nc.vector.tensor_tensor_scan(
                h1[:], f1[:], g1[:], 0.0, ALU.mult, ALU.subtract)
            nc.vector.tensor_tensor_scan(
                h2[:][:, ::-1], f2[:][:, ::-1], g2[:][:, ::-1],
                0.0, ALU.mult, ALU.subtract)
            nc.vector.tensor_add(
                ob[:, :, wc],
                h1[:].rearrange("p (j cd) -> p j cd", j=HB),
                h2[:].rearrange("p (j cd) -> p j cd", j=HB))

        def load_half(h):
            xb = xbufs[h % NXS]
            nc.sync.dma_start(xb[0:48], xhalf[h % (B * NHALF), 0:48])
            nc.sync.dma_start(xb[64:112], xhalf[h % (B * NHALF), 48:96])

        def hid_of(blk):
            return (blk // (NBLK // B)) * NHALF + (blk % (NBLK // B))

        nblk_tot = reps * NBLK
        nhid_tot = reps * B * NHALF
        # prologue loads ONLY the two halves block 0 needs; halves 2-3
        # queue after block 0's emission so the first conv isn't stuck
        # behind 3MB of DMA at the slow per-queue rate
        next_hid = 0
        while next_hid <= min(1, nhid_tot - 1):
            load_half(next_hid)
            next_hid += 1
        n_grp_tot = 2 * ((D + DG - 1) // DG)   # evacs per block

        def emit_scans(bc):
            ob = ob_pool.tile([128, HB, 2, HID * D], F16, tag="ob",
                              name="ob")
            for wc in range(2):
                scan_chunk(bc["Ts"][wc], ob, wc)
            dst = out[bc["s0"]:bc["s0"] + HB * W].rearrange(
                "(jq p) c d -> p jq (c d)", p=128)
            nc.gpsimd.dma_start(dst, ob[:].rearrange(
                "p j q cd -> p (j q) cd"))

        def emit_tp(bc, dd, wc):
            # regular matmul against identity: out[w, (j,c)] =
            # ev[(j,c), w]; strided psum dst makes the group's column
            # order (j, c, d) so evac+scan are contiguous
            g = dd // DG
            kk = dd % DG
            tps = bc["tps"]
            if (wc, g) not in tps:
                tps[(wc, g)] = tp_pool.tile(
                    [128, DG * 96], F32, tag="tp", name="tp")
            tpv = tps[(wc, g)][:].rearrange(
                "p (j c d) -> p j c d", j=HB, c=CO)
            # one accumulation group per (wc, g): start only on the first
            # d-slice, stop on the last, so the ~53ns array drain between
            # independent psum groups pipelines away (disjoint columns, so
            # accumulate == plain write)
            nc.tensor.matmul(
                tpv[:, :, :, kk],
                bc["ev"][:, dd, wc * 128:(wc + 1) * 128],
                isb[0:128, 0:96],
                start=(kk == 0), stop=(kk == DG - 1 or dd == D - 1),
                skip_group_check=True)
            if kk == DG - 1 or dd == D - 1:
                dk = kk + 1
                dst = bc["Tvs"][wc][:, :, :, g * DG:g * DG + dk]
                src = tps[(wc, g)][:].rearrange(
                    "p (j c d) -> p j c d", j=HB, c=CO)[:, :, :, 0:dk]
                if wc == 0:
                    nc.vector.tensor_copy(dst, src)
                else:
                    nc.scalar.activation(dst, src, AF.Copy)
                bc["nevac"] += 1
                if bc["nevac"] == n_grp_tot:
                    emit_scans(bc)

        # transposes lag their d-slice by 2 conv tiles (so their
        # LDWEIGHTS hide under conv streams and the tanh is done); the
        # queue persists across blocks so block tails interleave into the
        # next block's conv stream
        pending = []
        for blk in range(nblk_tot):
            hid = hid_of(blk % NBLK) + (blk // NBLK) * B * NHALF
            s0 = (blk % NBLK) * HB * W
            ev = evbufs[blk % 2]
            Ts = [t_pool.tile([128, HB * CD], F16, tag=f"T{wc}",
                              name=f"T{wc}")
                  for wc in range(2)]
            bc = {"ev": ev, "Ts": Ts,
                  "Tvs": [T[:].rearrange("p (j c d) -> p j c d",
                                         j=HB, c=CO) for T in Ts],
                  "tps": {}, "nevac": 0, "s0": s0}
            for dc in range(n_dc):
                gdc = blk * n_dc + dc
                d0 = dc * DC
                dn = min(DC, D - d0)
                ps = ps_pool.tile([2 * CO, DC * W], F32, tag="ps")
                psv = ps[:, 0:dn * W].rearrange("p (d w) -> p d w", w=W)
                k = 0
                for p in range(2):
                    xb = xbufs[(hid + p) % NXS]
                    for kw in range(3):
                        nc.tensor.matmul(
                            psv,
                            wsb[0:112, k * 96:(k + 1) * 96],
                            xb[0:112, d0:d0 + dn, kw:kw + W],
                            start=(k == 0), stop=(k == NST - 1))
                        lag = 1 if blk == nblk_tot - 1 else 2
                        if k >= 2 and pending and \
                                pending[0][0] <= gdc - lag:
                            _, pbc, dd, wc = pending.pop(0)
                            emit_tp(pbc, dd, wc)
                        k += 1
                evv = ev[0:96, d0:d0 + dn, :].rearrange("p d w -> p (d w)")
                nc.scalar.activation(evv, ps[:, 0:dn * W], AF.Tanh)
                for dd in range(d0, d0 + dn):
                    for wc in range(2):
                        pending.append((gdc, bc, dd, wc))
            while next_hid < nhid_tot and next_hid <= hid + 3:
                load_half(next_hid)
                next_hid += 1
        for _, pbc, dd, wc in pending:
            emit_tp(pbc, dd, wc)

    nc.finalize()
    return nc


def _host_inputs(x, Wc, b):
    """x: [B, CIN, D, H, W] f32 full input. Returns list of 8 in_maps."""
    bf = np.float16
    # 6 stationaries: idx = p*3+kw, each [128, 96] with cols (j*48+co).
    # rows 0-47 (block A, x at tile-h 2p):   tap kh = 2p - j
    # rows 64-111 (block B, x at h+1):       tap kh = 2p + 1 - j
    wt = np.zeros((NST, 128, 2 * CO), np.float32)
    for p in range(2):
        for kw in range(3):
            idx = p * 3 + kw
            for j in range(2):
                c0 = j * CO
                for blk, khv in ((0, 2 * p - j), (64, 2 * p + 1 - j)):
                    if khv < 0 or khv > 2:
                        continue
                    for kd in range(3):
                        p0 = blk + kd * 16
                        wt[idx, p0:p0 + 16, c0:c0 + CO] = \
                            Wc[:, :, kd, khv, kw].T
    wt[0, 48, 0:CO] = b
    wt[0, 48, CO:2 * CO] = b
    # pre-scale F1/F2 columns (weights and bias) by 0.5 so the single
    # Tanh evac yields t with sigmoid(a) = (t+1)/2
    for j in range(2):
        wt[:, :, j * CO + HID:j * CO + 3 * HID] *= 0.5
    wts = wt.transpose(1, 0, 2).reshape(128, NST * 2 * CO).astype(bf)
    auxa = np.zeros((16, FX), np.float32)
    auxa[0, :] = 1.0
    auxa = auxa.astype(bf)
    identa = np.zeros((128, 96), bf)
    identa[0:96, 0:96] = np.eye(96, dtype=bf)

    xt = np.ascontiguousarray(x.transpose(1, 2, 0, 3, 4)).astype(bf)
    in_maps = []
    for c in range(N_CORES):
        hs, he = c * HSH, (c + 1) * HSH
        xp = np.zeros((CIN, D + 2, B, HSH + 2, WP), bf)
        lo = max(hs - 1, 0)
        hi = min(he + 1, H)
        xp[:, 1:D + 1, :, (lo - (hs - 1)):(hi - (hs - 1)), 1:W + 1] = \
            xt[:, :, :, lo:hi, :]
        # pack x half-tiles: [B*NHALF, 96, D, WP], half q of batch b:
        # rows kd*16+ci    = xp[ci, kd+d, b, 2q, w]     (A: plane 2q-1)
        # rows 48+kd*16+ci = xp[ci, kd+d, b, 2q+1, w]   (B: plane 2q)
        xbk = np.empty((B, NHALF, 96, D, WP), bf)
        for kd in range(3):
            sl = xp[:, kd:kd + D]            # [CIN, D, B, HSH+2, WP]
            qa = np.arange(NHALF) * 2
            arr = sl[:, :, :, qa, :].transpose(2, 3, 0, 1, 4)
            xbk[:, :, kd * 16:kd * 16 + 16] = arr
            arr = sl[:, :, :, qa + 1, :].transpose(2, 3, 0, 1, 4)
            xbk[:, :, 48 + kd * 16:48 + kd * 16 + 16] = arr
        xbk = xbk.reshape(B * NHALF, 96, D, WP)
        in_maps.append({"x": xbk, "wts": wts, "aux": auxa, "ident": identa})
    return in_maps


_PROGRAM = None


def _get_program():
    global _PROGRAM
    if _PROGRAM is None:
        _PROGRAM = _build_program()
    return _PROGRAM


def run_sharded(in_maps, trace=False, **kw):
    from concourse import bass_utils
    nc = _get_program()
    return bass_utils.run_bass_kernel_spmd(
        nc, in_maps, core_ids=list(range(N_CORES)), trace=trace, **kw)


def _assemble(results):
    outf = np.empty((B, HID, D, H, W), np.float32)
    for c in range(N_CORES):
        raw = np.asarray(results[c]["out"]).astype(np.float32) * 0.5
        o = raw.reshape(B, HSH, W, HID, D).transpose(0, 3, 4, 1, 2)
        outf[:, :, :, c * HSH:(c + 1) * HSH, :] = o
    return outf


def kernel(x, W, b):
    x = np.asarray(x, np.float32)
    W = np.asarray(W, np.float32)
    b = np.asarray(b, np.float32)
    in_maps = _host_inputs(x, W, b)
    res = run_sharded(in_maps)
    return _assemble(res.results)
